# revision 54
# baseline (speedup 1.0000x reference)
"""Trainium2 Bass kernel for nn_ReachabilityClassifierTransformer.

Data-parallel over batch: 16 samples / 8 cores = 2 samples per core.
Each core runs the full network (6-layer encoder + 4-layer decoder + head)
on its 2 samples. No collectives.

Device layout conventions (per core):
  - Activations are kept FEATURE-MAJOR in SBUF: tile [128, KC, T] holds
    X.T, i.e. element [p, k, t] = X[t, k*128+p]. T = 2*512 tokens
    (sample-major concat).
  - All weights are pre-transposed on host to [in_feat, out_feat], cast to
    bf16, and laid out as [128, KC_in, O] (partition = in-feature % 128).
  - matmul(out_psum[M,N], lhsT=[K,M], rhs=[K,N]) computes lhsT.T @ rhs with
    K on partitions. bf16 operands run at full PE rate and enable FWL
    (fast weight load), so LDWEIGHTS overlaps the matmul stream.
  - The residual streams (x, p) stay float32r; LN statistics contract them
    against a ones vector on the PE at full rate.
  - V projection is fused on host: V = h @ (W1v_folded.T @ Wv.T) + vb_eff,
    removing the stage-1 v matmuls entirely (the MHA re-projection quirk
    composes two linear maps).
  - Encoder stage-1 q,k output features are de-interleaved (even feats then
    odd feats) via host-side column permutation of in_proj, so RoPE becomes
    contiguous block ops; the roped result is in natural order again.
  - Softmax: scores are computed transposed (S.T = K_h @ Q_h.T per 128-row
    chunk), exp'd without max subtraction (|scores/8| < 1 for this model),
    and the denominator comes free from a ones-column appended to V.
    Reciprocal uses the fast approximate custom-DVE op; the per-token
    normalizer is broadcast across partitions on the (otherwise idle)
    GPSIMD engine instead of the tensor engine.
"""
import numpy as np
import ml_dtypes

import concourse.bass as bass
import concourse.mybir as mybir
import concourse.tile as tile
from concourse import bacc
from concourse.bass_utils import run_bass_kernel_spmd

AF = mybir.ActivationFunctionType
ALU = mybir.AluOpType
F32 = mybir.dt.float32
F32R = mybir.dt.float32r
BF = mybir.dt.bfloat16
FP8 = mybir.dt.float8e4
DR = mybir.MatmulPerfMode.DoubleRow
BF_NP = ml_dtypes.bfloat16
FP8_NP = ml_dtypes.float8_e4m3
WS = 64.0          # fp8 weight pre-scale (keeps 0.02-scale weights normal)
WSI = 1.0 / WS

import os
# fp8 DoubleRow paths exist but are disabled: measured end-to-end rel-err
# was 1.2e-2 (ffn) / 1.8e-2 (all) vs 5e-4 in bf16 — too close to the 2e-2
# gate. Flip via K_FP8=attn,ffn,dec only for experiments.
_FP8_PARTS = set(os.environ.get("K_FP8", "none").split(","))
FP8_ATTN = "attn" in _FP8_PARTS   # encoder stage1/V/out-proj path
FP8_FFN = "ffn" in _FP8_PARTS     # encoder FFN
FP8_DEC = "dec" in _FP8_PARTS     # me + decoder K/V projections

B, S, D, FF, H, LE, LD, M = 16, 512, 512, 2048, 8, 6, 4, 2048
ROPE_BASE = 10000.0
LN_EPS = 1e-5
NCORES = 8
BL = B // NCORES          # 2 samples per core
T = BL * S                # 1024 tokens per core
KC = D // 128             # 4 feature chunks
FC = FF // 128            # 16
MC = M // 128             # 16
DH = D // H               # 64


# ----------------------------------------------------------------------------
# host-side helpers
# ----------------------------------------------------------------------------

def _chunked(wT, dt=BF_NP):
    """[Din, O] -> [128, Din//128, O] contiguous."""
    Din, O = wT.shape
    return np.ascontiguousarray(
        wT.reshape(Din // 128, 128, O).transpose(1, 0, 2)).astype(dt)


def _chunked8(wT):
    """fp8 weight, pre-scaled by WS: [Din, O] -> [128, Din//128, O]."""
    return _chunked(np.asarray(wT) * WS, FP8_NP)


def _bias_cols(b):
    """[O] -> [128, O//128]  (column per 128-chunk)."""
    O = b.shape[0]
    return np.ascontiguousarray(b.reshape(O // 128, 128).T).astype(np.float32)


_DEINT = np.concatenate([np.arange(0, D, 2), np.arange(1, D, 2)])  # de-interleave


def prep_weights(inp, le=LE, ld=LD):
    """Host-side weight prep -> dict of arrays shared by all cores."""
    out = {}
    g = {k: np.asarray(v, np.float32) for k, v in inp.items()}

    def _cattn(wT):
        return _chunked8(wT) if FP8_ATTN else _chunked(wT)

    def _cffn(wT):
        return _chunked8(wT) if FP8_FFN else _chunked(wT)

    out["mpwT"] = np.ascontiguousarray(g["morph_proj_w"].T)        # [3, 512]
    out["mpb"] = _bias_cols(g["morph_proj_b"])                     # [128, 4]
    out["ppwT"] = np.ascontiguousarray(g["pose_proj_w"].T)         # [9, 512]
    out["ppb"] = _bias_cols(g["pose_proj_b"])

    # rope grids, de-interleaved frequency order: [128, 2, 512]
    freq = 1.0 / ROPE_BASE ** (np.arange(0, D, 2, dtype=np.float64) / D)
    ang = np.outer(np.arange(S, dtype=np.float64), freq)           # [512, 256]
    out["gridc"] = _chunked(np.cos(ang).T.astype(np.float32).reshape(256, S))
    out["grids"] = _chunked(np.sin(ang).T.astype(np.float32).reshape(256, S))

    e_w1, e_w1b, e_w2, e_w2b, e_wv, e_vb = [], [], [], [], [], []
    e_ow, e_owb, e_l1, e_l1b, e_l2, e_l2b = [], [], [], [], [], []
    perm = np.concatenate([_DEINT, D + _DEINT])
    for i in range(le):
        w1 = g["enc_in_w"][i] * g["enc_n1_g"][i][None, :]          # fold n1 g
        b1 = g["enc_in_b"][i] + g["enc_in_w"][i] @ g["enc_n1_b"][i]
        # stage-1 q,k: de-interleave output columns (fp8, pre-scaled)
        e_w1.append(_cattn(np.ascontiguousarray(w1[:2 * D][perm].T)))
        e_w1b.append(_bias_cols(b1[:2 * D][perm]))                 # [128,8]
        # stage-2 q,k (natural order, raw weights - the faithful quirk)
        w2 = g["enc_in_w"][i][: 2 * D]                             # Wq;Wk
        e_w2.append(_chunked(np.ascontiguousarray(w2.T)))          # [128,4,1024]
        e_w2b.append(_bias_cols(g["enc_in_b"][i][: 2 * D]))        # [128,8]
        # fused V: V = h @ (W1v_f.T @ Wv.T) + vb_eff (fp8, pre-scaled)
        w1v_f = w1[2 * D:]                                         # [D, D] folded
        wv_raw = g["enc_in_w"][i][2 * D:]                          # [D, D]
        e_wv.append(_cattn(np.ascontiguousarray(w1v_f.T @ wv_raw.T)))
        vb_eff = b1[2 * D:] @ wv_raw.T + g["enc_in_b"][i][2 * D:]
        e_vb.append(vb_eff[None, :].astype(np.float32))            # [1,512]
        e_ow.append(_cattn(np.ascontiguousarray(g["enc_out_w"][i].T)))
        e_owb.append(_bias_cols(g["enc_out_b"][i]))
        l1 = g["enc_l1_w"][i] * g["enc_n2_g"][i][None, :]
        l1b = g["enc_l1_b"][i] + g["enc_l1_w"][i] @ g["enc_n2_b"][i]
        e_l1.append(_cffn(np.ascontiguousarray(l1.T)))         # [128,4,2048]
        e_l1b.append(_bias_cols(l1b))                              # [128,16]
        e_l2.append(_cffn(np.ascontiguousarray(g["enc_l2_w"][i].T)))
        e_l2b.append(_bias_cols(g["enc_l2_b"][i]))                 # [128,4]
    out["ew1T"], out["ew1b"] = np.stack(e_w1), np.stack(e_w1b)
    out["ew2T"], out["ew2b"] = np.stack(e_w2), np.stack(e_w2b)
    out["ewvT"], out["evb"] = np.stack(e_wv), np.stack(e_vb)
    out["eowT"], out["eowb"] = np.stack(e_ow), np.stack(e_owb)
    out["el1T"], out["el1b"] = np.stack(e_l1), np.stack(e_l1b)
    out["el2T"], out["el2b"] = np.stack(e_l2), np.stack(e_l2b)

    d_inq, d_inkv, d_inb, d_vb, d_ow, d_owb = [], [], [], [], [], []
    d_m1, d_m1b, d_m2, d_m2b = [], [], [], []
    for i in range(ld):
        w = g["dec_in_w"][i].copy()
        b = g["dec_in_b"][i].copy()
        w[:D] = w[:D] * g["dec_n1_g"][i][None, :]                  # Wq <- dec_n1
        b[:D] = b[:D] + g["dec_in_w"][i][:D] @ g["dec_n1_b"][i]
        w[D:] = w[D:] * g["enc_final_g"][None, :]                  # Wk,Wv <- enc_final
        b[D:] = b[D:] + g["dec_in_w"][i][D:] @ g["enc_final_b"]
        d_inq.append(_chunked(np.ascontiguousarray(w[:D].T)))      # [128,4,512] bf16
        d_inkv.append((_chunked8 if FP8_DEC else _chunked)(np.ascontiguousarray(w[D:].T)))    # [128,4,1024] fp8
        d_inb.append(_bias_cols(b))
        d_vb.append(b[2 * D:][None, :].astype(np.float32))         # [1,512]
        d_ow.append(_chunked(np.ascontiguousarray(g["dec_out_w"][i].T)))
        d_owb.append(_bias_cols(g["dec_out_b"][i]))
        m1 = g["dec_m1_w"][i] * g["dec_n2_g"][i][None, :]
        m1b = g["dec_m1_b"][i] + g["dec_m1_w"][i] @ g["dec_n2_b"][i]
        d_m1.append(_chunked(np.ascontiguousarray(m1.T)))          # [128,4,2048]
        d_m1b.append(_bias_cols(m1b))
        d_m2.append(_chunked(np.ascontiguousarray(g["dec_m2_w"][i].T)))
        d_m2b.append(_bias_cols(g["dec_m2_b"][i]))
    out["dinqT"], out["dinkvT"] = np.stack(d_inq), np.stack(d_inkv)
    out["dinb"] = np.stack(d_inb)
    out["dvb"] = np.stack(d_vb)
    out["dowT"], out["dowb"] = np.stack(d_ow), np.stack(d_owb)
    out["dm1T"], out["dm1b"] = np.stack(d_m1), np.stack(d_m1b)
    out["dm2T"], out["dm2b"] = np.stack(d_m2), np.stack(d_m2b)

    hw = (g["head_w"] * g["head_g"][None, :])[0]                   # [512]
    out["hwT"] = _bias_cols(hw).astype(BF_NP)                      # [128, 4]
    out["hb"] = (g["head_bias"] + g["head_w"] @ g["head_b"]).reshape(1, 1)
    return out


# ----------------------------------------------------------------------------
# device program
# ----------------------------------------------------------------------------

def build(le=LE, ld=LD, dbg=False):
    nc = bacc.Bacc(None, target_bir_lowering=False)

    dram = {}

    def din(name, shape, dt=BF):
        dram[name] = nc.dram_tensor(name, list(shape), dt, kind="ExternalInput")
        return dram[name]

    # shared weights
    din("mpwT", [3, 512], F32R); din("mpb", [128, 4], F32)
    din("ppwT", [9, 512], F32R); din("ppb", [128, 4], F32)
    din("gridc", [128, 2, S]); din("grids", [128, 2, S])
    din("ew1T", [le, 128, KC, 2 * D], FP8 if FP8_ATTN else BF); din("ew1b", [le, 128, 8], F32)
    din("ew2T", [le, 128, KC, 2 * D]); din("ew2b", [le, 128, 8], F32)
    din("ewvT", [le, 128, KC, D], FP8 if FP8_ATTN else BF); din("evb", [le, 1, D], F32)
    din("eowT", [le, 128, KC, D], FP8 if FP8_ATTN else BF); din("eowb", [le, 128, 4], F32)
    din("el1T", [le, 128, KC, FF], FP8 if FP8_FFN else BF); din("el1b", [le, 128, 16], F32)
    din("el2T", [le, 128, FC, D], FP8 if FP8_FFN else BF); din("el2b", [le, 128, 4], F32)
    din("dinqT", [ld, 128, KC, D]); din("dinkvT", [ld, 128, KC, 2 * D], FP8 if FP8_DEC else BF)
    din("dinb", [ld, 128, 12], F32)
    din("dvb", [ld, 1, D], F32)
    din("dowT", [ld, 128, KC, D]); din("dowb", [ld, 128, 4], F32)
    din("dm1T", [ld, 128, KC, M]); din("dm1b", [ld, 128, 16], F32)
    din("dm2T", [ld, 128, MC, D]); din("dm2b", [ld, 128, 4], F32)
    din("hwT", [128, KC]); din("hb", [1, 1], F32)
    # per-core inputs
    din("morphT", [3, T], F32R)
    din("poseT", [9, BL], F32R)
    y = nc.dram_tensor("y", [1, BL], F32, kind="ExternalOutput")

    dbg_t = {}
    if dbg:
        for i in range(le + 1):
            dbg_t[f"dx{i}"] = nc.dram_tensor(
                f"dx{i}", [128, KC, T], F32R, kind="ExternalOutput")
        dbg_t["dme"] = nc.dram_tensor("dme", [128, KC, T], FP8 if FP8_DEC else BF, kind="ExternalOutput")
        for i in range(ld + 1):
            dbg_t[f"dp{i}"] = nc.dram_tensor(
                f"dp{i}", [128, KC, BL], F32R, kind="ExternalOutput")

    with tile.TileContext(nc) as tc:
        _build_body(nc, tc, dram, y, le, ld, dbg_t)
    nc.compile()
    return nc


def _build_body(nc, tc, dram, y_dram, le, ld, dbg_t=None):
    dbg_t = dbg_t or {}
    import contextlib
    ctx = contextlib.ExitStack()
    with ctx:
        ctx.enter_context(nc.allow_low_precision(
            reason="bf16 matmul operands and f32r stats are intentional"))
        persist = ctx.enter_context(tc.tile_pool(name="persist", bufs=1))
        wpool = ctx.enter_context(tc.tile_pool(name="wpool", bufs=2))
        w2pool = ctx.enter_context(tc.tile_pool(name="w2pool", bufs=2))
        owpool = ctx.enter_context(tc.tile_pool(name="owpool", bufs=2))
        bpool = ctx.enter_context(tc.tile_pool(name="bpool", bufs=2))
        a4 = ctx.enter_context(tc.tile_pool(name="a4", bufs=4))
        a8 = ctx.enter_context(tc.tile_pool(name="a8", bufs=3))
        vp = ctx.enter_context(tc.tile_pool(name="vp", bufs=2))
        rtp = ctx.enter_context(tc.tile_pool(name="rtp", bufs=3))
        vbp = ctx.enter_context(tc.tile_pool(name="vbp", bufs=1))
        scr = ctx.enter_context(tc.tile_pool(name="scr", bufs=4))
        scrrc = ctx.enter_context(tc.tile_pool(name="scrrc", bufs=1))
        smalls = ctx.enter_context(tc.tile_pool(name="smalls", bufs=2))
        b1 = ctx.enter_context(tc.tile_pool(name="b1", bufs=4, space="PSUM"))
        b2 = ctx.enter_context(tc.tile_pool(name="b2", bufs=4, space="PSUM"))
        qk2p, atp = a8, a4  # share slots/tags
        AT8 = FP8 if FP8_ATTN else BF
        FF8 = FP8 if FP8_FFN else BF
        DC8 = FP8 if FP8_DEC else BF
        A_SC = WSI if FP8_ATTN else 1.0
        D_SC = WSI if FP8_DEC else 1.0

        # ---------------- persistent tiles ----------------
        x = persist.tile([128, KC, T], F32R)          # residual stream (X.T)
        me = persist.tile([128, KC, T], DC8)          # final-LN'd encoder out
        gridc = persist.tile([128, 2, S], BF)
        grids = persist.tile([128, 2, S], BF)
        ones128 = persist.tile([128, 1], F32R)
        ones128b = persist.tile([128, 1], BF)
        ones8 = persist.tile([128, 8], BF)
        eps_t = persist.tile([1, 1], F32)
        p = persist.tile([128, KC, BL], F32R)         # decoder latent p.T
        nc.sync.dma_start(gridc[:], dram["gridc"][:])
        nc.sync.dma_start(grids[:], dram["grids"][:])
        stage_f32 = rtp.tile([128, 128], F32, tag="rt")
        nc.vector.memset(stage_f32[:], 1.0)
        nc.vector.tensor_copy(ones128[:], stage_f32[:, 0:1])
        nc.vector.tensor_copy(ones128b[:], stage_f32[:, 0:1])
        nc.vector.tensor_copy(ones8[:], stage_f32[:, 0:8])
        nc.vector.memset(eps_t[:], LN_EPS)

        def c32(ap):
            return ap.bitcast(F32)

        def wmm(ps, wt, msl, rhs, fp8):
            """ps += wt[:, :, msl].T @ rhs, DoubleRow pairs when fp8."""
            if fp8:
                for kp in range(KC // 2):
                    nc.tensor.matmul(ps, wt[:, 2 * kp:2 * kp + 2, msl],
                                     rhs[:, 2 * kp:2 * kp + 2, :], perf_mode=DR,
                                     start=(kp == 0), stop=(kp == KC // 2 - 1))
            else:
                for k in range(KC):
                    nc.tensor.matmul(ps, wt[:, k, msl], rhs[:, k, :],
                                     start=(k == 0), stop=(k == KC - 1))


        def ln(x_tile, n_tok, h_out):
            """h_out = LayerNorm_features(x_tile), small-N variant (decoder)."""
            sq = a4.tile([128, KC, n_tok], BF, tag="sq_d")
            for k in range(KC):
                nc.scalar.activation(sq[:, k, :], x_tile[:, k, :], AF.Square)
            sum_ps = b2.tile([1, n_tok], F32, tag="b2")
            sq_ps = b2.tile([1, n_tok], F32, tag="b2")
            for k in range(KC):
                nc.tensor.matmul(sum_ps[:], c32(ones128[:]), c32(x_tile[:, k, :]),
                                 start=(k == 0), stop=(k == KC - 1))
            for k in range(KC):
                nc.tensor.matmul(sq_ps[:], ones128b[:], sq[:, k, :],
                                 start=(k == 0), stop=(k == KC - 1))
            t2 = scr.tile([1, n_tok], F32, tag="scr")
            rc = scrrc.tile([1, 2, n_tok], F32R, tag="scr_rc")
            mneg = rc[:, 1, :]
            nc.scalar.activation(mneg, sum_ps[:], AF.Copy, scale=-1.0 / D)
            nc.vector.tensor_tensor(t2[:], mneg, mneg, ALU.mult)        # m^2
            nc.vector.scalar_tensor_tensor(
                t2[:], sq_ps[:], 1.0 / D, t2[:], ALU.mult, ALU.subtract)
            nc.scalar.activation(t2[:], t2[:], AF.Sqrt, bias=eps_t[:])
            nc.vector.reciprocal_approx_fast(rc[:, 0, :].bitcast(F32), t2[:])
            r_bc = scr.tile([128, n_tok], F32R, tag="scr_bc")
            m_bc = scr.tile([128, n_tok], F32R, tag="scr_bc")
            nc.gpsimd.partition_broadcast(r_bc[:], rc[:, 0, :])
            nc.gpsimd.partition_broadcast(m_bc[:], rc[:, 1, :])
            for k in range(KC):
                tmp = smalls.tile([128, n_tok], BF, tag="lnt")
                nc.vector.tensor_tensor(tmp[:], x_tile[:, k, :],
                                        m_bc[:], ALU.add)
                nc.vector.tensor_tensor(h_out[:, k, :], tmp[:],
                                        r_bc[:], ALU.mult)

        def ln_stats(x_tile, s):
            """LN stats for sample s: rc [1, 2, S] (r, -m) f32r."""
            sl = slice(s * S, (s + 1) * S)
            sq = a8.tile([128, KC, S], BF, tag="a8")
            for k in range(KC):
                nc.scalar.activation(sq[:, k, :], x_tile[:, k, sl], AF.Square)
            sum_ps = b2.tile([1, S], F32, tag="b2")
            sq_ps = b2.tile([1, S], F32, tag="b2")
            for k in range(KC):
                nc.tensor.matmul(sum_ps[:], ones128[:], x_tile[:, k, sl],
                                 start=(k == 0), stop=(k == KC - 1))
            for k in range(KC):
                nc.tensor.matmul(sq_ps[:], ones128b[:], sq[:, k, :],
                                 start=(k == 0), stop=(k == KC - 1))
            rc = scrrc.tile([1, 2, S], BF, tag="scr_rcT", bufs=4)
            ms = scr.tile([1, S], F32, tag="scr")
            t2 = scrrc.tile([1, S], F32, tag="scrT2", bufs=1)
            nc.scalar.activation(ms[:], sum_ps[:], AF.Copy, scale=-1.0 / D)
            nc.vector.tensor_copy(rc[:, 1, :], ms[:])
            nc.vector.tensor_tensor(t2[:], ms[:], ms[:], ALU.mult)
            nc.vector.scalar_tensor_tensor(
                t2[:], sq_ps[:], 1.0 / D, t2[:], ALU.mult, ALU.subtract)
            nc.scalar.activation(t2[:], t2[:], AF.Sqrt, bias=eps_t[:])
            rf = scr.tile([1, S], F32, tag="scr")
            nc.vector.reciprocal_approx_fast(rf[:], t2[:])
            nc.vector.tensor_copy(rc[:, 0, :], rf[:])
            r_bc = scr.tile([128, S], BF, tag="scr_bcS", bufs=8)
            m_bc = scr.tile([128, S], BF, tag="scr_bcS", bufs=8)
            nc.gpsimd.partition_broadcast(r_bc[:], rc[:, 0, :])
            nc.gpsimd.partition_broadcast(m_bc[:], rc[:, 1, :])
            return (r_bc, m_bc)

        def ln_apply(bcs, x_tile, s, h_out):
            sl = slice(s * S, (s + 1) * S)
            r_bc, m_bc = bcs
            for k in range(KC):
                tmp = smalls.tile([128, S], BF, tag="lnt")
                nc.vector.tensor_tensor(tmp[:], x_tile[:, k, sl],
                                        m_bc[:], ALU.add)
                nc.vector.tensor_tensor(h_out[:, k, :], tmp[:],
                                        r_bc[:], ALU.mult)

        # ---------------- morph projection -> x ----------------
        morpht = a8.tile([3, T], F32R, tag="a8")
        nc.sync.dma_start(morpht[:], dram["morphT"][:])
        mpw = rtp.tile([3, 512], F32R, tag="rt")
        mpb = rtp.tile([128, 4], F32, tag="rt")
        nc.sync.dma_start(mpw[:], dram["mpwT"][:])
        nc.sync.dma_start(mpb[:], dram["mpb"][:])
        for m in range(KC):
            for s in range(BL):
                ps = b1.tile([128, S], F32, tag="b1")
                nc.tensor.matmul(ps[:], mpw[:, m * 128:(m + 1) * 128],
                                 morpht[:, s * S:(s + 1) * S], start=True, stop=True)
                nc.scalar.activation(x[:, m, s * S:(s + 1) * S], ps[:], AF.Relu,
                                     bias=mpb[:, m:m + 1])

        rcA = [None] * BL
        for s in range(BL):
            rcA[s] = ln_stats(x, s)

        # ---------------- pose projection -> p ----------------
        poset = rtp.tile([9, BL], F32R, tag="rt")
        ppw = rtp.tile([9, 512], F32R, tag="rt")
        ppb = rtp.tile([128, 4], F32, tag="rt")
        nc.sync.dma_start(poset[:], dram["poseT"][:])
        nc.sync.dma_start(ppw[:], dram["ppwT"][:])
        nc.sync.dma_start(ppb[:], dram["ppb"][:])
        pps = b1.tile([128, KC, BL], F32, tag="b1")
        for m in range(KC):
            nc.tensor.matmul(pps[:, m, :], c32(ppw[:, m * 128:(m + 1) * 128]),
                             c32(poset[:]), start=True, stop=True)
        for m in range(KC):
            nc.scalar.activation(p[:, m, :], pps[:, m, :], AF.Relu,
                                 bias=ppb[:, m:m + 1])

        def dump(name, tile_ap):
            if name in dbg_t:
                nc.sync.dma_start(dbg_t[name][:], tile_ap)

        dump("dx0", x[:])

        # ---------------- encoder layers ----------------
        for li in range(le):
            w1 = wpool.tile([128, KC, 2 * D], FP8, tag="bigw")
            nc.sync.dma_start(w1[:], dram["ew1T"][li])
            w1b = bpool.tile([128, 8], F32, tag="w1b")
            nc.sync.dma_start(w1b[:], dram["ew1b"][li])
            w2 = w2pool.tile([128, KC, 2 * D], BF, tag="w2")
            nc.sync.dma_start(w2[:], dram["ew2T"][li])
            w2b = bpool.tile([128, 8], F32, tag="w2b")
            nc.sync.dma_start(w2b[:], dram["ew2b"][li])
            wv = owpool.tile([128, KC, D], FP8, tag="wv")
            nc.sync.dma_start(wv[:], dram["ewvT"][li])
            vbrow = bpool.tile([1, D], F32, tag="vbrow")
            nc.sync.dma_start(vbrow[:], dram["evb"][li])
            ow = owpool.tile([128, KC, D], FP8, tag="ow")
            nc.sync.dma_start(ow[:], dram["eowT"][li])
            owb = bpool.tile([128, 4], F32, tag="owb")
            nc.sync.dma_start(owb[:], dram["eowb"][li])
            l1 = wpool.tile([128, KC, FF], FP8, tag="bigw")
            nc.sync.dma_start(l1[:], dram["el1T"][li])
            l1b = bpool.tile([128, 16], F32, tag="l1b")
            nc.sync.dma_start(l1b[:], dram["el1b"][li])
            l2b = bpool.tile([128, 4], F32, tag="l2b")
            nc.sync.dma_start(l2b[:], dram["el2b"][li])

            # v-bias broadcast [128, 512] (token-major V bias), once per layer
            vb_bc = vbp.tile([128, D], F32, tag="vb_bc")
            nc.gpsimd.partition_broadcast(vb_bc[:], vbrow[:])

            rcB = [None] * BL
            for s in range(BL):
                sl = slice(s * S, (s + 1) * S)
                h = a4.tile([128, KC, S], AT8, tag="a4h")
                ln_apply(rcA[s], x, s, h)
                # ---- stage 1: q,k (permuted, de-interleaved) ----
                qkv1 = a8.tile([128, 8, S], BF, tag="a8")
                for m in range(8):
                    ps = b1.tile([128, S], F32, tag="b1")
                    wmm(ps[:], w1, slice(m * 128, (m + 1) * 128), h, FP8_ATTN)
                    nc.vector.tensor_scalar(qkv1[:, m, :], ps[:],
                                            A_SC, w1b[:, m:m + 1], ALU.mult, ALU.add)
                # ---- rope: qkv1 -> qkr (natural order) ----
                qkr = a8.tile([128, 8, S], BF, tag="a8")
                for half in (0, 4):
                    for c in range(2):
                        e = qkv1[:, half + c, :]
                        o = qkv1[:, half + 2 + c, :]
                        r1 = qkr[:, half + c, :]
                        r2 = qkr[:, half + 2 + c, :]
                        t1 = rtp.tile([128, S], BF, tag="rt")
                        nc.vector.tensor_tensor(r1, e, gridc[:, c, :], ALU.mult)
                        nc.vector.tensor_tensor(t1[:], o, grids[:, c, :], ALU.mult)
                        nc.vector.tensor_tensor(r1, r1, t1[:], ALU.subtract)
                        t2 = rtp.tile([128, S], BF, tag="rt")
                        nc.vector.tensor_tensor(r2, e, grids[:, c, :], ALU.mult)
                        nc.vector.tensor_tensor(t2[:], o, gridc[:, c, :], ALU.mult)
                        nc.vector.tensor_tensor(r2, r2, t2[:], ALU.add)
                # ---- stage 2: Q,K ----
                qk2 = qk2p.tile([128, 8, S], BF, tag="a8")
                for m in range(8):
                    ps = b1.tile([128, S], F32, tag="b1")
                    base = 0 if m < 4 else 4
                    for k in range(KC):
                        nc.tensor.matmul(ps[:], w2[:, k, m * 128:(m + 1) * 128],
                                         qkr[:, base + k, :],
                                         start=(k == 0), stop=(k == KC - 1))
                    nc.vector.tensor_scalar(qk2[:, m, :], ps[:],
                                            w2b[:, m:m + 1], None, ALU.add)
                # ---- fused V (token-major, with ones column per head) ----
                vloc = vp.tile([128, KC, 8, 65], BF, tag="vloc")
                for t in range(KC):
                    nc.vector.tensor_copy(vloc[:, t, :, 64], ones8[:])
                for t in range(KC):
                    ps = b1.tile([128, S], F32, tag="b1")
                    if FP8_ATTN:
                        for kp in range(KC // 2):
                            nc.tensor.matmul(
                                ps[:], h[:, 2 * kp:2 * kp + 2, t * 128:(t + 1) * 128],
                                wv[:, 2 * kp:2 * kp + 2, :], perf_mode=DR,
                                start=(kp == 0), stop=(kp == KC // 2 - 1))
                    else:
                        for k in range(KC):
                            nc.tensor.matmul(
                                ps[:], h[:, k, t * 128:(t + 1) * 128], wv[:, k, :],
                                start=(k == 0), stop=(k == KC - 1))
                    nc.vector.scalar_tensor_tensor(
                        vloc[:, t, :, 0:64],
                        ps[:].rearrange("p (h d) -> p h d", h=H), A_SC,
                        vb_bc[:].rearrange("p (h d) -> p h d", h=H),
                        ALU.mult, ALU.add)
                # ---- attention heads (paired: exp(h+1) hides under A@V(h)) ----
                o_t = a4.tile([128, KC, S], AT8, tag="a4h")
                for h0 in range(0, H, 2):
                    ats = {}
                    for hh in (h0, h0 + 1):
                        ats[hh] = atp.tile([128, KC, S], BF, tag="a4",
                                           name=f"at_{hh}")
                    for c in range(KC):
                        for hh in (h0, h0 + 1):
                            rows = slice(64 * (hh % 2), 64 * (hh % 2) + 64)
                            scp = b1.tile([128, S], F32, tag="b1")
                            nc.tensor.matmul(
                                scp[:],
                                qk2[rows, 4 + hh // 2, c * 128:(c + 1) * 128],
                                qk2[rows, hh // 2, :], start=True, stop=True)
                            nc.scalar.activation(ats[hh][:, c, :], scp[:], AF.Exp,
                                                 scale=float(1.0 / np.sqrt(DH)))
                    for hh in (h0, h0 + 1):
                        rows = slice(64 * (hh % 2), 64 * (hh % 2) + 64)
                        at = ats[hh]
                        ov = b2.tile([65, S], F32, tag="b2")
                        for c in range(KC):
                            nc.tensor.matmul(ov[:], vloc[:, c, hh, :], at[:, c, :],
                                             start=(c == 0), stop=(c == KC - 1))
                        den = scr.tile([1, S], F32, tag="scr")
                        nc.vector.tensor_copy(den[:], ov[64:65, :])
                        rec = scr.tile([1, S], F32, tag="scr")
                        nc.vector.reciprocal_approx_fast(rec[:], den[:])
                        rb = scr.tile([64, S], F32, tag="scr_rb")
                        nc.gpsimd.partition_broadcast(rb[:], rec[:])
                        nc.vector.tensor_tensor(o_t[rows, hh // 2, :],
                                                ov[0:64, :], rb[:], ALU.mult)
                # ---- out-proj (fp8 DoubleRow) + residual ----
                for m in range(KC):
                    ps = b1.tile([128, S], F32, tag="b1")
                    wmm(ps[:], ow, slice(m * 128, (m + 1) * 128), o_t, FP8_ATTN)
                    if FP8_ATTN:
                        ot_sb = rtp.tile([128, S], BF, tag="rt")
                        nc.vector.tensor_scalar(ot_sb[:], ps[:], A_SC,
                                                owb[:, m:m + 1], ALU.mult, ALU.add)
                        nc.vector.tensor_tensor(x[:, m, sl], ot_sb[:], x[:, m, sl],
                                                ALU.add)
                    else:
                        nc.vector.scalar_tensor_tensor(
                            x[:, m, sl], ps[:], owb[:, m:m + 1], x[:, m, sl],
                            ALU.add, ALU.add)
                rcB[s] = ln_stats(x, s)
            # ---- phase B: l2 streams in (chunked) once w1 is released ----
            l2 = wpool.tile([128, FC, D], FP8, tag="bigw")
            for kf in range(FC):
                nc.sync.dma_start(l2[:, kf, :], dram["el2T"][li][:, kf, :])
            for s in range(BL):
                sl = slice(s * S, (s + 1) * S)
                h2 = a4.tile([128, KC, S], FF8, tag="a4h")
                ln_apply(rcB[s], x, s, h2)
                # ---- FFN ----
                f2 = [b1.tile([128, S], F32, tag="b1", name=f"f2_{_m}")
                      for _m in range(KC)]
                if FP8_FFN:
                    for kfp in range(FC // 2):
                        rt2 = rtp.tile([128, 2, S], FP8, tag="rt")
                        for j in (0, 1):
                            kf = 2 * kfp + j
                            f1 = b2.tile([128, S], F32, tag="b2")
                            wmm(f1[:], l1, slice(kf * 128, (kf + 1) * 128), h2, True)
                            nc.scalar.activation(rt2[:, j, :], f1[:], AF.Relu,
                                                 bias=l1b[:, kf:kf + 1], scale=WSI)
                        for m in range(KC):
                            nc.tensor.matmul(
                                f2[m][:],
                                l2[:, 2 * kfp:2 * kfp + 2, m * 128:(m + 1) * 128],
                                rt2[:], perf_mode=DR,
                                start=(kfp == 0), stop=(kfp == FC // 2 - 1))
                    for m in range(KC):
                        f2sb = rtp.tile([128, S], BF, tag="rt")
                        nc.vector.tensor_scalar(f2sb[:], f2[m][:], WSI,
                                                l2b[:, m:m + 1], ALU.mult, ALU.add)
                        nc.vector.tensor_tensor(x[:, m, sl], f2sb[:], x[:, m, sl],
                                                ALU.add)
                else:
                    for kf in range(FC):
                        f1 = b2.tile([128, S], F32, tag="b2")
                        wmm(f1[:], l1, slice(kf * 128, (kf + 1) * 128), h2, False)
                        rt = rtp.tile([128, S], BF, tag="rt")
                        nc.scalar.activation(rt[:], f1[:], AF.Relu,
                                             bias=l1b[:, kf:kf + 1])
                        for m in range(KC):
                            nc.tensor.matmul(f2[m][:],
                                             l2[:, kf, m * 128:(m + 1) * 128],
                                             rt[:], start=(kf == 0),
                                             stop=(kf == FC - 1))
                    for m in range(KC):
                        nc.vector.scalar_tensor_tensor(
                            x[:, m, sl], f2[m][:], l2b[:, m:m + 1], x[:, m, sl],
                            ALU.add, ALU.add)
                rcA[s] = ln_stats(x, s)
                if li == le - 1:
                    ln_apply(rcA[s], x, s, me[:, :, s * S:(s + 1) * S])
            dump(f"dx{li + 1}", x[:])

        # final encoder LN -> me is emitted inside the last FFN phase above
        dump("dme", me[:])
        dump("dp0", p[:])

        # ---------------- decoder layers ----------------
        for li in range(ld):
            dwq = w2pool.tile([128, KC, D], BF, tag="w2")
            nc.sync.dma_start(dwq[:], dram["dinqT"][li])
            dw = wpool.tile([128, KC, 2 * D], FP8, tag="bigw")
            nc.sync.dma_start(dw[:], dram["dinkvT"][li])
            dwb = bpool.tile([128, 12], F32, tag="w1b")
            nc.sync.dma_start(dwb[:], dram["dinb"][li])
            dvbrow = bpool.tile([1, D], F32, tag="vbrow")
            nc.sync.dma_start(dvbrow[:], dram["dvb"][li])
            do = owpool.tile([128, KC, D], BF, tag="ow")
            nc.sync.dma_start(do[:], dram["dowT"][li])
            dob = bpool.tile([128, 4], F32, tag="owb")
            nc.sync.dma_start(dob[:], dram["dowb"][li])
            m1 = wpool.tile([128, KC, M], BF, tag="bigw")
            nc.sync.dma_start(m1[:], dram["dm1T"][li])
            m1b = bpool.tile([128, 16], F32, tag="l1b")
            nc.sync.dma_start(m1b[:], dram["dm1b"][li])
            m2b = bpool.tile([128, 4], F32, tag="l2b")
            nc.sync.dma_start(m2b[:], dram["dm2b"][li])

            vb_bc = vbp.tile([128, D], F32, tag="vb_bc")
            nc.gpsimd.partition_broadcast(vb_bc[:], dvbrow[:])

            # LN(p) -> q_ln ; Q projection (all samples at once, N=BL)
            q_ln = smalls.tile([128, KC, BL], BF, tag="q_ln")
            ln(p, BL, q_ln)
            qps = b1.tile([128, KC, BL], F32, tag="b1")
            for m in range(KC):
                for k in range(KC):
                    nc.tensor.matmul(qps[:, m, :],
                                     dwq[:, k, m * 128:(m + 1) * 128],
                                     q_ln[:, k, :], start=(k == 0),
                                     stop=(k == KC - 1))
            q_sb = smalls.tile([128, KC, BL], BF, tag="q_sb")
            for m in range(KC):
                nc.vector.tensor_scalar(q_sb[:, m, :], qps[:, m, :],
                                        dwb[:, m:m + 1], None, ALU.add)
            o_d = smalls.tile([128, KC, BL], BF, tag="o_d")
            for s in range(BL):
                sl = slice(s * S, (s + 1) * S)
                # K (feature-major) and V' (token-major) over morph_enc
                k_sb = a4.tile([128, KC, S], BF, tag="a4")
                me_s = me[:, :, sl]
                for m in range(KC):
                    ps = b1.tile([128, S], F32, tag="b1")
                    wmm(ps[:], dw, slice(m * 128, (m + 1) * 128), me_s, FP8_DEC)
                    nc.vector.tensor_scalar(k_sb[:, m, :], ps[:], D_SC,
                                            dwb[:, 4 + m:5 + m], ALU.mult, ALU.add)
                vloc = vp.tile([128, KC, 8, 65], BF, tag="vloc")
                for t in range(KC):
                    nc.vector.tensor_copy(vloc[:, t, :, 64], ones8[:])
                for t in range(KC):
                    ps = b1.tile([128, S], F32, tag="b1")
                    tsl = slice(s * S + t * 128, s * S + (t + 1) * 128)
                    if FP8_DEC:
                        for kp in range(KC // 2):
                            nc.tensor.matmul(
                                ps[:], me[:, 2 * kp:2 * kp + 2, tsl],
                                dw[:, 2 * kp:2 * kp + 2, D:2 * D], perf_mode=DR,
                                start=(kp == 0), stop=(kp == KC // 2 - 1))
                    else:
                        for k in range(KC):
                            nc.tensor.matmul(
                                ps[:], me[:, k, tsl], dw[:, k, D:2 * D],
                                start=(k == 0), stop=(k == KC - 1))
                    nc.vector.scalar_tensor_tensor(
                        vloc[:, t, :, 0:64],
                        ps[:].rearrange("p (h d) -> p h d", h=H), D_SC,
                        vb_bc[:].rearrange("p (h d) -> p h d", h=H),
                        ALU.mult, ALU.add)
                scp = b1.tile([128, KC, H], F32, tag="b1")
                for hh in range(H):
                    rows = slice(64 * (hh % 2), 64 * (hh % 2) + 64)
                    for c in range(KC):
                        nc.tensor.matmul(
                            scp[:, c, hh:hh + 1],
                            k_sb[rows, hh // 2, c * 128:(c + 1) * 128],
                            q_sb[rows, hh // 2, s:s + 1],
                            start=True, stop=True)
                at = smalls.tile([128, KC, H], BF, tag="at_d")
                nc.scalar.activation(at[:], scp[:], AF.Exp,
                                     scale=float(1.0 / np.sqrt(DH)))
                ov = b2.tile([65, H], F32, tag="b2")
                for hh in range(H):
                    for c in range(KC):
                        nc.tensor.matmul(ov[:, hh:hh + 1], vloc[:, c, hh, :],
                                         at[:, c, hh:hh + 1],
                                         start=(c == 0), stop=(c == KC - 1))
                den_d = scr.tile([1, H], F32, tag="scr")
                nc.vector.tensor_copy(den_d[:], ov[64:65, :])
                rec = scr.tile([1, H], F32, tag="scr")
                nc.vector.reciprocal_approx_fast(rec[:], den_d[:])
                rb = scr.tile([64, H], F32, tag="scr_rb")
                nc.gpsimd.partition_broadcast(rb[:], rec[:])
                for hh in range(H):
                    rows = slice(64 * (hh % 2), 64 * (hh % 2) + 64)
                    nc.vector.tensor_tensor(o_d[rows, hh // 2, s:s + 1],
                                            ov[0:64, hh:hh + 1],
                                            rb[:, hh:hh + 1], ALU.mult)
            # out-proj + residual into p
            ops = b1.tile([128, KC, BL], F32, tag="b1")
            for m in range(KC):
                for k in range(KC):
                    nc.tensor.matmul(ops[:, m, :],
                                     do[:, k, m * 128:(m + 1) * 128],
                                     o_d[:, k, :], start=(k == 0),
                                     stop=(k == KC - 1))
            for m in range(KC):
                nc.vector.scalar_tensor_tensor(
                    p[:, m, :], ops[:, m, :], dob[:, m:m + 1], p[:, m, :],
                    ALU.add, ALU.add)
            # FFN on p (m2 streams in chunked once dw releases its slot)
            m2 = wpool.tile([128, MC, D], BF, tag="bigw")
            for kf in range(MC):
                nc.sync.dma_start(m2[:, kf, :], dram["dm2T"][li][:, kf, :])
            h2d = smalls.tile([128, KC, BL], BF, tag="q_ln")
            ln(p, BL, h2d)
            mh = smalls.tile([128, MC, BL], BF, tag="mh")
            for mm_ in range(MC):
                ps = b1.tile([128, BL], F32, tag="b1")
                for k in range(KC):
                    nc.tensor.matmul(ps[:], m1[:, k, mm_ * 128:(mm_ + 1) * 128],
                                     h2d[:, k, :], start=(k == 0),
                                     stop=(k == KC - 1))
                nc.scalar.activation(mh[:, mm_, :], ps[:], AF.Relu,
                                     bias=m1b[:, mm_:mm_ + 1])
            m2ps = b1.tile([128, KC, BL], F32, tag="b1")
            for m in range(KC):
                for kf in range(MC):
                    nc.tensor.matmul(m2ps[:, m, :],
                                     m2[:, kf, m * 128:(m + 1) * 128],
                                     mh[:, kf, :], start=(kf == 0),
                                     stop=(kf == MC - 1))
            for m in range(KC):
                nc.vector.scalar_tensor_tensor(
                    p[:, m, :], m2ps[:, m, :], m2b[:, m:m + 1], p[:, m, :],
                    ALU.add, ALU.add)
            dump(f"dp{li + 1}", p[:])

        # ---------------- head ----------------
        hw = smalls.tile([128, KC], BF, tag="hw")
        hb = smalls.tile([1, 1], F32, tag="hb")
        nc.sync.dma_start(hw[:], dram["hwT"][:])
        nc.sync.dma_start(hb[:], dram["hb"][:])
        hg = smalls.tile([128, KC, BL], BF, tag="q_ln")
        ln(p, BL, hg)
        hps = b2.tile([1, BL], F32, tag="b2")
        for k in range(KC):
            nc.tensor.matmul(hps[:], hw[:, k:k + 1], hg[:, k, :],
                             start=(k == 0), stop=(k == KC - 1))
        y_sb = smalls.tile([1, BL], F32, tag="y_sb")
        nc.scalar.activation(y_sb[:], hps[:], AF.Sigmoid, bias=hb[:])
        nc.sync.dma_start(y_dram[:], y_sb[:])


# ----------------------------------------------------------------------------
# entry point
# ----------------------------------------------------------------------------

_NC_CACHE = {}


def kernel(**inputs):
    return _run(inputs, LE, LD)


def _run(inputs, le, ld, trace=False):
    w = prep_weights(inputs, le, ld)
    morph = np.asarray(inputs["morph"], np.float32)
    pose = np.asarray(inputs["pose"], np.float32)
    in_maps = []
    for c in range(NCORES):
        im = dict(w)
        mo = morph[c * BL:(c + 1) * BL]                 # [BL, S, 3]
        im["morphT"] = np.ascontiguousarray(
            mo.transpose(2, 0, 1).reshape(3, T))
        im["poseT"] = np.ascontiguousarray(pose[c * BL:(c + 1) * BL].T)
        in_maps.append(im)

    if ("nc", le, ld) not in _NC_CACHE:
        _NC_CACHE[("nc", le, ld)] = build(le, ld)
    nc = _NC_CACHE[("nc", le, ld)]
    res = run_bass_kernel_spmd(nc, in_maps, core_ids=list(range(NCORES)),
                               trace=trace)
    out = np.zeros((B, 1), np.float32)
    for c in range(NCORES):
        out[c * BL:(c + 1) * BL, 0] = res.results[c]["y"][0]
    if trace:
        return out, res
    return out


# revision 55
# speedup vs baseline: 1.2063x; 1.2063x over previous
"""Trainium2 Bass kernel for nn_ReachabilityClassifierTransformer.

Data-parallel over batch: 16 samples / 8 cores = 2 samples per core.
Each core runs the full network (6-layer encoder + 4-layer decoder + head)
on its 2 samples. No collectives.

Device layout conventions (per core):
  - Activations are kept FEATURE-MAJOR in SBUF: tile [128, KC, T] holds
    X.T, i.e. element [p, k, t] = X[t, k*128+p]. T = 2*512 tokens
    (sample-major concat).
  - All weights are pre-transposed on host to [in_feat, out_feat], cast to
    bf16, and laid out as [128, KC_in, O] (partition = in-feature % 128).
  - matmul(out_psum[M,N], lhsT=[K,M], rhs=[K,N]) computes lhsT.T @ rhs with
    K on partitions. bf16 operands run at full PE rate and enable FWL
    (fast weight load), so LDWEIGHTS overlaps the matmul stream.
  - The residual streams (x, p) stay float32r; LN statistics contract them
    against a ones vector on the PE at full rate.
  - V projection is fused on host: V = h @ (W1v_folded.T @ Wv.T) + vb_eff,
    removing the stage-1 v matmuls entirely (the MHA re-projection quirk
    composes two linear maps).
  - Encoder stage-1 q,k output features are de-interleaved (even feats then
    odd feats) via host-side column permutation of in_proj, so RoPE becomes
    contiguous block ops; the roped result is in natural order again.
  - Softmax: scores are computed transposed (S.T = K_h @ Q_h.T per 128-row
    chunk), exp'd without max subtraction (|scores/8| < 1 for this model),
    and the denominator comes free from a ones-column appended to V.
    Reciprocal uses the fast approximate custom-DVE op; the per-token
    normalizer is broadcast across partitions on the (otherwise idle)
    GPSIMD engine instead of the tensor engine.
"""
import numpy as np
import ml_dtypes

import concourse.bass as bass
import concourse.mybir as mybir
import concourse.tile as tile
from concourse import bacc
from concourse.bass_utils import run_bass_kernel_spmd

AF = mybir.ActivationFunctionType
ALU = mybir.AluOpType
F32 = mybir.dt.float32
F32R = mybir.dt.float32r
BF = mybir.dt.bfloat16
FP8 = mybir.dt.float8e4
DR = mybir.MatmulPerfMode.DoubleRow
BF_NP = ml_dtypes.bfloat16
FP8_NP = ml_dtypes.float8_e4m3
WS = 64.0          # fp8 weight pre-scale (keeps 0.02-scale weights normal)
WSI = 1.0 / WS

import os
# fp8 DoubleRow paths exist but are disabled: measured end-to-end rel-err
# was 1.2e-2 (ffn) / 1.8e-2 (all) vs 5e-4 in bf16 — too close to the 2e-2
# gate. Flip via K_FP8=attn,ffn,dec only for experiments.
_FP8_PARTS = set(os.environ.get("K_FP8", "none").split(","))
FP8_ATTN = "attn" in _FP8_PARTS   # encoder stage1/V/out-proj path
FP8_FFN = "ffn" in _FP8_PARTS     # encoder FFN
FP8_DEC = "dec" in _FP8_PARTS     # me + decoder K/V projections

B, S, D, FF, H, LE, LD, M = 16, 512, 512, 2048, 8, 6, 4, 2048
ROPE_BASE = 10000.0
LN_EPS = 1e-5
NCORES = 8
BL = B // NCORES          # 2 samples per core
T = BL * S                # 1024 tokens per core
KC = D // 128             # 4 feature chunks
FC = FF // 128            # 16
MC = M // 128             # 16
DH = D // H               # 64


# ----------------------------------------------------------------------------
# host-side helpers
# ----------------------------------------------------------------------------

def _chunked(wT, dt=BF_NP):
    """[Din, O] -> [128, Din//128, O] contiguous."""
    Din, O = wT.shape
    return np.ascontiguousarray(
        wT.reshape(Din // 128, 128, O).transpose(1, 0, 2)).astype(dt)


def _chunked8(wT):
    """fp8 weight, pre-scaled by WS: [Din, O] -> [128, Din//128, O]."""
    return _chunked(np.asarray(wT) * WS, FP8_NP)


def _bias_cols(b):
    """[O] -> [128, O//128]  (column per 128-chunk)."""
    O = b.shape[0]
    return np.ascontiguousarray(b.reshape(O // 128, 128).T).astype(np.float32)


_DEINT = np.concatenate([np.arange(0, D, 2), np.arange(1, D, 2)])  # de-interleave


def prep_weights(inp, le=LE, ld=LD):
    """Host-side weight prep -> dict of arrays shared by all cores."""
    out = {}
    g = {k: np.asarray(v, np.float32) for k, v in inp.items()}

    def _cattn(wT):
        return _chunked8(wT) if FP8_ATTN else _chunked(wT)

    def _cffn(wT):
        return _chunked8(wT) if FP8_FFN else _chunked(wT)

    out["mpwT"] = np.ascontiguousarray(g["morph_proj_w"].T)        # [3, 512]
    out["mpb"] = _bias_cols(g["morph_proj_b"])                     # [128, 4]
    out["ppwT"] = np.ascontiguousarray(g["pose_proj_w"].T)         # [9, 512]
    out["ppb"] = _bias_cols(g["pose_proj_b"])

    # rope grids, de-interleaved frequency order: [128, 2, 512]
    freq = 1.0 / ROPE_BASE ** (np.arange(0, D, 2, dtype=np.float64) / D)
    ang = np.outer(np.arange(S, dtype=np.float64), freq)           # [512, 256]
    out["gridc"] = _chunked(np.cos(ang).T.astype(np.float32).reshape(256, S))
    out["grids"] = _chunked(np.sin(ang).T.astype(np.float32).reshape(256, S))

    e_w1, e_w1b, e_w2, e_w2b, e_wv, e_vb = [], [], [], [], [], []
    e_ow, e_owb, e_l1, e_l1b, e_l2, e_l2b = [], [], [], [], [], []
    perm = np.concatenate([_DEINT, D + _DEINT])
    for i in range(le):
        w1 = g["enc_in_w"][i] * g["enc_n1_g"][i][None, :]          # fold n1 g
        b1 = g["enc_in_b"][i] + g["enc_in_w"][i] @ g["enc_n1_b"][i]
        # stage-1 q,k: de-interleave output columns (fp8, pre-scaled)
        e_w1.append(_cattn(np.ascontiguousarray(w1[:2 * D][perm].T)))
        e_w1b.append(_bias_cols(b1[:2 * D][perm]))                 # [128,8]
        # stage-2 q,k (natural order, raw weights - the faithful quirk)
        w2 = g["enc_in_w"][i][: 2 * D]                             # Wq;Wk
        e_w2.append(_chunked(np.ascontiguousarray(w2.T)))          # [128,4,1024]
        e_w2b.append(_bias_cols(g["enc_in_b"][i][: 2 * D]))        # [128,8]
        # fused V: V = h @ (W1v_f.T @ Wv.T) + vb_eff (fp8, pre-scaled)
        w1v_f = w1[2 * D:]                                         # [D, D] folded
        wv_raw = g["enc_in_w"][i][2 * D:]                          # [D, D]
        e_wv.append(_cattn(np.ascontiguousarray(w1v_f.T @ wv_raw.T)))
        vb_eff = b1[2 * D:] @ wv_raw.T + g["enc_in_b"][i][2 * D:]
        e_vb.append(vb_eff[None, :].astype(np.float32))            # [1,512]
        e_ow.append(_cattn(np.ascontiguousarray(g["enc_out_w"][i].T)))
        e_owb.append(_bias_cols(g["enc_out_b"][i]))
        l1 = g["enc_l1_w"][i] * g["enc_n2_g"][i][None, :]
        l1b = g["enc_l1_b"][i] + g["enc_l1_w"][i] @ g["enc_n2_b"][i]
        e_l1.append(_cffn(np.ascontiguousarray(l1.T)))         # [128,4,2048]
        e_l1b.append(_bias_cols(l1b))                              # [128,16]
        e_l2.append(_cffn(np.ascontiguousarray(g["enc_l2_w"][i].T)))
        e_l2b.append(_bias_cols(g["enc_l2_b"][i]))                 # [128,4]
    out["ew1T"], out["ew1b"] = np.stack(e_w1), np.stack(e_w1b)
    out["ew2T"], out["ew2b"] = np.stack(e_w2), np.stack(e_w2b)
    out["ewvT"], out["evb"] = np.stack(e_wv), np.stack(e_vb)
    out["eowT"], out["eowb"] = np.stack(e_ow), np.stack(e_owb)
    out["el1T"], out["el1b"] = np.stack(e_l1), np.stack(e_l1b)
    out["el2T"], out["el2b"] = np.stack(e_l2), np.stack(e_l2b)

    d_inq, d_inkv, d_inb, d_vb, d_ow, d_owb = [], [], [], [], [], []
    d_m1, d_m1b, d_m2, d_m2b = [], [], [], []
    for i in range(ld):
        w = g["dec_in_w"][i].copy()
        b = g["dec_in_b"][i].copy()
        w[:D] = w[:D] * g["dec_n1_g"][i][None, :]                  # Wq <- dec_n1
        b[:D] = b[:D] + g["dec_in_w"][i][:D] @ g["dec_n1_b"][i]
        w[D:] = w[D:] * g["enc_final_g"][None, :]                  # Wk,Wv <- enc_final
        b[D:] = b[D:] + g["dec_in_w"][i][D:] @ g["enc_final_b"]
        d_inq.append(_chunked(np.ascontiguousarray(w[:D].T)))      # [128,4,512] bf16
        d_inkv.append((_chunked8 if FP8_DEC else _chunked)(np.ascontiguousarray(w[D:].T)))    # [128,4,1024] fp8
        d_inb.append(_bias_cols(b))
        d_vb.append(b[2 * D:][None, :].astype(np.float32))         # [1,512]
        d_ow.append(_chunked(np.ascontiguousarray(g["dec_out_w"][i].T)))
        d_owb.append(_bias_cols(g["dec_out_b"][i]))
        m1 = g["dec_m1_w"][i] * g["dec_n2_g"][i][None, :]
        m1b = g["dec_m1_b"][i] + g["dec_m1_w"][i] @ g["dec_n2_b"][i]
        d_m1.append(_chunked(np.ascontiguousarray(m1.T)))          # [128,4,2048]
        d_m1b.append(_bias_cols(m1b))
        d_m2.append(_chunked(np.ascontiguousarray(g["dec_m2_w"][i].T)))
        d_m2b.append(_bias_cols(g["dec_m2_b"][i]))
    out["dinqT"], out["dinkvT"] = np.stack(d_inq), np.stack(d_inkv)
    out["dinb"] = np.stack(d_inb)
    out["dvb"] = np.stack(d_vb)
    out["dowT"], out["dowb"] = np.stack(d_ow), np.stack(d_owb)
    out["dm1T"], out["dm1b"] = np.stack(d_m1), np.stack(d_m1b)
    out["dm2T"], out["dm2b"] = np.stack(d_m2), np.stack(d_m2b)

    hw = (g["head_w"] * g["head_g"][None, :])[0]                   # [512]
    out["hwT"] = _bias_cols(hw).astype(BF_NP)                      # [128, 4]
    out["hb"] = (g["head_bias"] + g["head_w"] @ g["head_b"]).reshape(1, 1)
    return out


# ----------------------------------------------------------------------------
# device program
# ----------------------------------------------------------------------------

def build(le=LE, ld=LD, dbg=False):
    nc = bacc.Bacc(None, target_bir_lowering=False)

    dram = {}

    def din(name, shape, dt=BF):
        dram[name] = nc.dram_tensor(name, list(shape), dt, kind="ExternalInput")
        return dram[name]

    # shared weights
    din("mpwT", [3, 512], F32R); din("mpb", [128, 4], F32)
    din("ppwT", [9, 512], F32R); din("ppb", [128, 4], F32)
    din("gridc", [128, 2, S]); din("grids", [128, 2, S])
    din("ew1T", [le, 128, KC, 2 * D], FP8 if FP8_ATTN else BF); din("ew1b", [le, 128, 8], F32)
    din("ew2T", [le, 128, KC, 2 * D]); din("ew2b", [le, 128, 8], F32)
    din("ewvT", [le, 128, KC, D], FP8 if FP8_ATTN else BF); din("evb", [le, 1, D], F32)
    din("eowT", [le, 128, KC, D], FP8 if FP8_ATTN else BF); din("eowb", [le, 128, 4], F32)
    din("el1T", [le, 128, KC, FF], FP8 if FP8_FFN else BF); din("el1b", [le, 128, 16], F32)
    din("el2T", [le, 128, FC, D], FP8 if FP8_FFN else BF); din("el2b", [le, 128, 4], F32)
    din("dinqT", [ld, 128, KC, D]); din("dinkvT", [ld, 128, KC, 2 * D], FP8 if FP8_DEC else BF)
    din("dinb", [ld, 128, 12], F32)
    din("dvb", [ld, 1, D], F32)
    din("dowT", [ld, 128, KC, D]); din("dowb", [ld, 128, 4], F32)
    din("dm1T", [ld, 128, KC, M]); din("dm1b", [ld, 128, 16], F32)
    din("dm2T", [ld, 128, MC, D]); din("dm2b", [ld, 128, 4], F32)
    din("hwT", [128, KC]); din("hb", [1, 1], F32)
    # per-core inputs
    din("morphT", [3, T], F32R)
    din("poseT", [9, BL], F32R)
    y = nc.dram_tensor("y", [1, BL], F32, kind="ExternalOutput")

    dbg_t = {}
    if dbg:
        for i in range(le + 1):
            dbg_t[f"dx{i}"] = nc.dram_tensor(
                f"dx{i}", [128, KC, T], F32R, kind="ExternalOutput")
        dbg_t["dme"] = nc.dram_tensor("dme", [128, KC, T], FP8 if FP8_DEC else BF, kind="ExternalOutput")
        for i in range(ld + 1):
            dbg_t[f"dp{i}"] = nc.dram_tensor(
                f"dp{i}", [128, KC, BL], F32R, kind="ExternalOutput")

    with tile.TileContext(nc) as tc:
        _build_body(nc, tc, dram, y, le, ld, dbg_t)
    nc.compile()
    return nc


def _build_body(nc, tc, dram, y_dram, le, ld, dbg_t=None):
    dbg_t = dbg_t or {}
    import contextlib
    ctx = contextlib.ExitStack()
    with ctx:
        ctx.enter_context(nc.allow_low_precision(
            reason="bf16 matmul operands and f32r stats are intentional"))
        persist = ctx.enter_context(tc.tile_pool(name="persist", bufs=1))
        wpool = ctx.enter_context(tc.tile_pool(name="wpool", bufs=2))
        w2pool = ctx.enter_context(tc.tile_pool(name="w2pool", bufs=2))
        owpool = ctx.enter_context(tc.tile_pool(name="owpool", bufs=2))
        bpool = ctx.enter_context(tc.tile_pool(name="bpool", bufs=2))
        a4 = ctx.enter_context(tc.tile_pool(name="a4", bufs=4))
        a8 = ctx.enter_context(tc.tile_pool(name="a8", bufs=3))
        vp = ctx.enter_context(tc.tile_pool(name="vp", bufs=2))
        rtp = ctx.enter_context(tc.tile_pool(name="rtp", bufs=3))
        vbp = ctx.enter_context(tc.tile_pool(name="vbp", bufs=1))
        scr = ctx.enter_context(tc.tile_pool(name="scr", bufs=4))
        scrrc = ctx.enter_context(tc.tile_pool(name="scrrc", bufs=1))
        smalls = ctx.enter_context(tc.tile_pool(name="smalls", bufs=2))
        b1 = ctx.enter_context(tc.tile_pool(name="b1", bufs=4, space="PSUM"))
        b2 = ctx.enter_context(tc.tile_pool(name="b2", bufs=4, space="PSUM"))
        qk2p, atp = a8, a4  # share slots/tags
        AT8 = FP8 if FP8_ATTN else BF
        FF8 = FP8 if FP8_FFN else BF
        DC8 = FP8 if FP8_DEC else BF
        A_SC = WSI if FP8_ATTN else 1.0
        D_SC = WSI if FP8_DEC else 1.0

        # ---------------- persistent tiles ----------------
        x = persist.tile([128, KC, T], F32R)          # residual stream (X.T)
        me = persist.tile([128, KC, T], DC8)          # final-LN'd encoder out
        gridc = persist.tile([128, 2, S], BF)
        grids = persist.tile([128, 2, S], BF)
        ones128 = persist.tile([128, 1], F32R)
        ones128b = persist.tile([128, 1], BF)
        ones8 = persist.tile([128, 8], BF)
        eps_t = persist.tile([1, 1], F32)
        p = persist.tile([128, KC, BL], F32R)         # decoder latent p.T
        nc.sync.dma_start(gridc[:], dram["gridc"][:])
        nc.sync.dma_start(grids[:], dram["grids"][:])
        stage_f32 = rtp.tile([128, 128], F32, tag="rt")
        nc.vector.memset(stage_f32[:], 1.0)
        nc.vector.tensor_copy(ones128[:], stage_f32[:, 0:1])
        nc.vector.tensor_copy(ones128b[:], stage_f32[:, 0:1])
        nc.vector.tensor_copy(ones8[:], stage_f32[:, 0:8])
        nc.vector.memset(eps_t[:], LN_EPS)

        def c32(ap):
            return ap.bitcast(F32)

        def wmm(ps, wt, msl, rhs, fp8):
            """ps += wt[:, :, msl].T @ rhs, DoubleRow pairs when fp8."""
            if fp8:
                for kp in range(KC // 2):
                    nc.tensor.matmul(ps, wt[:, 2 * kp:2 * kp + 2, msl],
                                     rhs[:, 2 * kp:2 * kp + 2, :], perf_mode=DR,
                                     start=(kp == 0), stop=(kp == KC // 2 - 1))
            else:
                for k in range(KC):
                    nc.tensor.matmul(ps, wt[:, k, msl], rhs[:, k, :],
                                     start=(k == 0), stop=(k == KC - 1))


        def ln(x_tile, n_tok, h_out):
            """h_out = LayerNorm_features(x_tile), small-N variant (decoder)."""
            sq = a4.tile([128, KC, n_tok], BF, tag="sq_d")
            for k in range(KC):
                nc.scalar.activation(sq[:, k, :], x_tile[:, k, :], AF.Square)
            sum_ps = b2.tile([1, n_tok], F32, tag="b2")
            sq_ps = b2.tile([1, n_tok], F32, tag="b2")
            for k in range(KC):
                nc.tensor.matmul(sum_ps[:], c32(ones128[:]), c32(x_tile[:, k, :]),
                                 start=(k == 0), stop=(k == KC - 1))
            for k in range(KC):
                nc.tensor.matmul(sq_ps[:], ones128b[:], sq[:, k, :],
                                 start=(k == 0), stop=(k == KC - 1))
            t2 = scr.tile([1, n_tok], F32, tag="scr")
            rc = scrrc.tile([1, 2, n_tok], F32R, tag="scr_rc")
            mneg = rc[:, 1, :]
            nc.scalar.activation(mneg, sum_ps[:], AF.Copy, scale=-1.0 / D)
            nc.vector.tensor_tensor(t2[:], mneg, mneg, ALU.mult)        # m^2
            nc.vector.scalar_tensor_tensor(
                t2[:], sq_ps[:], 1.0 / D, t2[:], ALU.mult, ALU.subtract)
            nc.scalar.activation(t2[:], t2[:], AF.Sqrt, bias=eps_t[:])
            nc.vector.reciprocal_approx_fast(rc[:, 0, :].bitcast(F32), t2[:])
            r_bc = scr.tile([128, n_tok], F32R, tag="scr_bc")
            m_bc = scr.tile([128, n_tok], F32R, tag="scr_bc")
            nc.gpsimd.partition_broadcast(r_bc[:], rc[:, 0, :])
            nc.gpsimd.partition_broadcast(m_bc[:], rc[:, 1, :])
            for k in range(KC):
                tmp = smalls.tile([128, n_tok], BF, tag="lnt")
                nc.vector.tensor_tensor(tmp[:], x_tile[:, k, :],
                                        m_bc[:], ALU.add)
                nc.vector.tensor_tensor(h_out[:, k, :], tmp[:],
                                        r_bc[:], ALU.mult)

        def ln_stats(x_tile, s):
            """LN stats for sample s: rc [1, 2, S] (r, -m) f32r."""
            sl = slice(s * S, (s + 1) * S)
            sq = a8.tile([128, KC, S], BF, tag="a8")
            for k in range(KC):
                nc.scalar.activation(sq[:, k, :], x_tile[:, k, sl], AF.Square)
            sum_ps = b2.tile([1, S], F32, tag="b2")
            sq_ps = b2.tile([1, S], F32, tag="b2")
            for k in range(KC):
                nc.tensor.matmul(sum_ps[:], ones128[:], x_tile[:, k, sl],
                                 start=(k == 0), stop=(k == KC - 1))
            for k in range(KC):
                nc.tensor.matmul(sq_ps[:], ones128b[:], sq[:, k, :],
                                 start=(k == 0), stop=(k == KC - 1))
            rc = scrrc.tile([1, 2, S], BF, tag="scr_rcT", bufs=4)
            ms = scr.tile([1, S], F32, tag="scr")
            t2 = scrrc.tile([1, S], F32, tag="scrT2", bufs=1)
            nc.scalar.activation(ms[:], sum_ps[:], AF.Copy, scale=-1.0 / D)
            nc.vector.tensor_copy(rc[:, 1, :], ms[:])
            nc.vector.tensor_tensor(t2[:], ms[:], ms[:], ALU.mult)
            nc.vector.scalar_tensor_tensor(
                t2[:], sq_ps[:], 1.0 / D, t2[:], ALU.mult, ALU.subtract)
            nc.scalar.activation(t2[:], t2[:], AF.Sqrt, bias=eps_t[:])
            rf = scr.tile([1, S], F32, tag="scr")
            nc.vector.reciprocal_approx_fast(rf[:], t2[:])
            nc.vector.tensor_copy(rc[:, 0, :], rf[:])
            r_bc = scr.tile([128, S], BF, tag="scr_bcS", bufs=8)
            m_bc = scr.tile([128, S], BF, tag="scr_bcS", bufs=8)
            nc.gpsimd.partition_broadcast(r_bc[:], rc[:, 0, :])
            nc.gpsimd.partition_broadcast(m_bc[:], rc[:, 1, :])
            return (r_bc, m_bc)

        def ln_apply(bcs, x_tile, s, h_out):
            sl = slice(s * S, (s + 1) * S)
            r_bc, m_bc = bcs
            for k in range(KC):
                tmp = smalls.tile([128, S], BF, tag="lnt")
                nc.vector.tensor_tensor(tmp[:], x_tile[:, k, sl],
                                        m_bc[:], ALU.add)
                nc.vector.tensor_tensor(h_out[:, k, :], tmp[:],
                                        r_bc[:], ALU.mult)

        # ---------------- morph projection -> x ----------------
        morpht = a8.tile([3, T], F32R, tag="a8")
        nc.sync.dma_start(morpht[:], dram["morphT"][:])
        mpw = rtp.tile([3, 512], F32R, tag="rt")
        mpb = rtp.tile([128, 4], F32, tag="rt")
        nc.sync.dma_start(mpw[:], dram["mpwT"][:])
        nc.sync.dma_start(mpb[:], dram["mpb"][:])
        for m in range(KC):
            for s in range(BL):
                ps = b1.tile([128, S], F32, tag="b1")
                nc.tensor.matmul(ps[:], mpw[:, m * 128:(m + 1) * 128],
                                 morpht[:, s * S:(s + 1) * S], start=True, stop=True)
                nc.scalar.activation(x[:, m, s * S:(s + 1) * S], ps[:], AF.Relu,
                                     bias=mpb[:, m:m + 1])

        rcA = [None] * BL
        for s in range(BL):
            rcA[s] = ln_stats(x, s)

        # ---------------- pose projection -> p ----------------
        poset = rtp.tile([9, BL], F32R, tag="rt")
        ppw = rtp.tile([9, 512], F32R, tag="rt")
        ppb = rtp.tile([128, 4], F32, tag="rt")
        nc.sync.dma_start(poset[:], dram["poseT"][:])
        nc.sync.dma_start(ppw[:], dram["ppwT"][:])
        nc.sync.dma_start(ppb[:], dram["ppb"][:])
        pps = b1.tile([128, KC, BL], F32, tag="b1")
        for m in range(KC):
            nc.tensor.matmul(pps[:, m, :], c32(ppw[:, m * 128:(m + 1) * 128]),
                             c32(poset[:]), start=True, stop=True)
        for m in range(KC):
            nc.scalar.activation(p[:, m, :], pps[:, m, :], AF.Relu,
                                 bias=ppb[:, m:m + 1])

        def dump(name, tile_ap):
            if name in dbg_t:
                nc.sync.dma_start(dbg_t[name][:], tile_ap)

        dump("dx0", x[:])

        # ---------------- encoder layers ----------------
        for li in range(le):
            w1 = wpool.tile([128, KC, 2 * D], FP8, tag="bigw")
            nc.sync.dma_start(w1[:], dram["ew1T"][li])
            w1b = bpool.tile([128, 8], F32, tag="w1b")
            nc.sync.dma_start(w1b[:], dram["ew1b"][li])
            w2 = w2pool.tile([128, KC, 2 * D], BF, tag="w2")
            nc.sync.dma_start(w2[:], dram["ew2T"][li])
            w2b = bpool.tile([128, 8], F32, tag="w2b")
            nc.sync.dma_start(w2b[:], dram["ew2b"][li])
            wv = owpool.tile([128, KC, D], FP8, tag="wv")
            nc.sync.dma_start(wv[:], dram["ewvT"][li])
            vbrow = bpool.tile([1, D], F32, tag="vbrow")
            nc.sync.dma_start(vbrow[:], dram["evb"][li])
            ow = owpool.tile([128, KC, D], FP8, tag="ow")
            nc.sync.dma_start(ow[:], dram["eowT"][li])
            owb = bpool.tile([128, 4], F32, tag="owb")
            nc.sync.dma_start(owb[:], dram["eowb"][li])
            l1 = wpool.tile([128, KC, FF], FP8, tag="bigw")
            nc.sync.dma_start(l1[:], dram["el1T"][li])
            l1b = bpool.tile([128, 16], F32, tag="l1b")
            nc.sync.dma_start(l1b[:], dram["el1b"][li])
            l2b = bpool.tile([128, 4], F32, tag="l2b")
            nc.sync.dma_start(l2b[:], dram["el2b"][li])

            # v-bias broadcast [128, 512] (token-major V bias), once per layer
            vb_bc = vbp.tile([128, D], F32, tag="vb_bc")
            nc.gpsimd.partition_broadcast(vb_bc[:], vbrow[:])

            rcB = [None] * BL
            for s in range(BL):
                sl = slice(s * S, (s + 1) * S)
                h = a4.tile([128, KC, S], AT8, tag="a4h")
                ln_apply(rcA[s], x, s, h)
                # ---- stage 1: q,k (permuted, de-interleaved) ----
                qkv1 = a8.tile([128, 8, S], BF, tag="a8")
                for m in range(8):
                    ps = b1.tile([128, S], F32, tag="b1")
                    wmm(ps[:], w1, slice(m * 128, (m + 1) * 128), h, FP8_ATTN)
                    nc.vector.tensor_scalar(qkv1[:, m, :], ps[:],
                                            A_SC, w1b[:, m:m + 1], ALU.mult, ALU.add)
                # ---- rope: qkv1 -> qkr (natural order) ----
                qkr = a8.tile([128, 8, S], BF, tag="a8")
                for half in (0, 4):
                    for c in range(2):
                        e = qkv1[:, half + c, :]
                        o = qkv1[:, half + 2 + c, :]
                        r1 = qkr[:, half + c, :]
                        r2 = qkr[:, half + 2 + c, :]
                        t1 = rtp.tile([128, S], BF, tag="rt")
                        nc.vector.tensor_tensor(r1, e, gridc[:, c, :], ALU.mult)
                        nc.vector.tensor_tensor(t1[:], o, grids[:, c, :], ALU.mult)
                        nc.vector.tensor_tensor(r1, r1, t1[:], ALU.subtract)
                        t2 = rtp.tile([128, S], BF, tag="rt")
                        nc.vector.tensor_tensor(r2, e, grids[:, c, :], ALU.mult)
                        nc.vector.tensor_tensor(t2[:], o, gridc[:, c, :], ALU.mult)
                        nc.vector.tensor_tensor(r2, r2, t2[:], ALU.add)
                # ---- stage 2: Q,K ----
                qk2 = qk2p.tile([128, 8, S], BF, tag="a8")
                for m in range(8):
                    ps = b1.tile([128, S], F32, tag="b1")
                    base = 0 if m < 4 else 4
                    for k in range(KC):
                        nc.tensor.matmul(ps[:], w2[:, k, m * 128:(m + 1) * 128],
                                         qkr[:, base + k, :],
                                         start=(k == 0), stop=(k == KC - 1))
                    nc.vector.tensor_scalar(qk2[:, m, :], ps[:],
                                            w2b[:, m:m + 1], None, ALU.add)
                # ---- fused V (token-major, with ones column per head) ----
                vloc = vp.tile([128, KC, 8, 65], BF, tag="vloc")
                for t in range(KC):
                    nc.vector.tensor_copy(vloc[:, t, :, 64], ones8[:])
                for t in range(KC):
                    ps = b1.tile([128, S], F32, tag="b1")
                    if FP8_ATTN:
                        for kp in range(KC // 2):
                            nc.tensor.matmul(
                                ps[:], h[:, 2 * kp:2 * kp + 2, t * 128:(t + 1) * 128],
                                wv[:, 2 * kp:2 * kp + 2, :], perf_mode=DR,
                                start=(kp == 0), stop=(kp == KC // 2 - 1))
                    else:
                        for k in range(KC):
                            nc.tensor.matmul(
                                ps[:], h[:, k, t * 128:(t + 1) * 128], wv[:, k, :],
                                start=(k == 0), stop=(k == KC - 1))
                    nc.vector.scalar_tensor_tensor(
                        vloc[:, t, :, 0:64],
                        ps[:].rearrange("p (h d) -> p h d", h=H), A_SC,
                        vb_bc[:].rearrange("p (h d) -> p h d", h=H),
                        ALU.mult, ALU.add)
                # ---- attention heads (paired: exp(h+1) hides under A@V(h)) ----
                o_t = a4.tile([128, KC, S], AT8, tag="a4h")
                for h0 in range(0, H, 2):
                    ats = {}
                    for hh in (h0, h0 + 1):
                        ats[hh] = atp.tile([128, KC, S], BF, tag="a4",
                                           name=f"at_{hh}")
                    for c in range(KC):
                        for hh in (h0, h0 + 1):
                            rows = slice(64 * (hh % 2), 64 * (hh % 2) + 64)
                            scp = b1.tile([128, S], F32, tag="b1")
                            nc.tensor.matmul(
                                scp[:],
                                qk2[rows, 4 + hh // 2, c * 128:(c + 1) * 128],
                                qk2[rows, hh // 2, :], start=True, stop=True)
                            nc.scalar.activation(ats[hh][:, c, :], scp[:], AF.Exp,
                                                 scale=float(1.0 / np.sqrt(DH)))
                    for hh in (h0, h0 + 1):
                        rows = slice(64 * (hh % 2), 64 * (hh % 2) + 64)
                        at = ats[hh]
                        ov = b2.tile([65, S], F32, tag="b2")
                        for c in range(KC):
                            nc.tensor.matmul(ov[:], vloc[:, c, hh, :], at[:, c, :],
                                             start=(c == 0), stop=(c == KC - 1))
                        den = scr.tile([1, S], F32, tag="scr")
                        nc.vector.tensor_copy(den[:], ov[64:65, :])
                        rec = scr.tile([1, S], F32, tag="scr")
                        nc.vector.reciprocal_approx_fast(rec[:], den[:])
                        rb = scr.tile([64, S], F32, tag="scr_rb")
                        nc.gpsimd.partition_broadcast(rb[:], rec[:])
                        nc.vector.tensor_tensor(o_t[rows, hh // 2, :],
                                                ov[0:64, :], rb[:], ALU.mult)
                # ---- out-proj (fp8 DoubleRow) + residual ----
                for m in range(KC):
                    ps = b1.tile([128, S], F32, tag="b1")
                    wmm(ps[:], ow, slice(m * 128, (m + 1) * 128), o_t, FP8_ATTN)
                    if FP8_ATTN:
                        ot_sb = rtp.tile([128, S], BF, tag="rt")
                        nc.vector.tensor_scalar(ot_sb[:], ps[:], A_SC,
                                                owb[:, m:m + 1], ALU.mult, ALU.add)
                        nc.vector.tensor_tensor(x[:, m, sl], ot_sb[:], x[:, m, sl],
                                                ALU.add)
                    else:
                        nc.vector.scalar_tensor_tensor(
                            x[:, m, sl], ps[:], owb[:, m:m + 1], x[:, m, sl],
                            ALU.add, ALU.add)
                rcB[s] = ln_stats(x, s)
            # ---- phase B: l2 streams in (chunked) once w1 is released ----
            l2 = wpool.tile([128, FC, D], FP8, tag="bigw")
            for kf in range(FC):
                nc.sync.dma_start(l2[:, kf, :], dram["el2T"][li][:, kf, :])
            for s in range(BL):
                sl = slice(s * S, (s + 1) * S)
                h2 = a4.tile([128, KC, S], FF8, tag="a4h")
                ln_apply(rcB[s], x, s, h2)
                # ---- FFN ----
                f2 = [b1.tile([128, S], F32, tag="b1", name=f"f2_{_m}")
                      for _m in range(KC)]
                if FP8_FFN:
                    for kfp in range(FC // 2):
                        rt2 = rtp.tile([128, 2, S], FP8, tag="rt")
                        for j in (0, 1):
                            kf = 2 * kfp + j
                            f1 = b2.tile([128, S], F32, tag="b2")
                            wmm(f1[:], l1, slice(kf * 128, (kf + 1) * 128), h2, True)
                            nc.scalar.activation(rt2[:, j, :], f1[:], AF.Relu,
                                                 bias=l1b[:, kf:kf + 1], scale=WSI)
                        for m in range(KC):
                            nc.tensor.matmul(
                                f2[m][:],
                                l2[:, 2 * kfp:2 * kfp + 2, m * 128:(m + 1) * 128],
                                rt2[:], perf_mode=DR,
                                start=(kfp == 0), stop=(kfp == FC // 2 - 1))
                    for m in range(KC):
                        f2sb = rtp.tile([128, S], BF, tag="rt")
                        nc.vector.tensor_scalar(f2sb[:], f2[m][:], WSI,
                                                l2b[:, m:m + 1], ALU.mult, ALU.add)
                        nc.vector.tensor_tensor(x[:, m, sl], f2sb[:], x[:, m, sl],
                                                ALU.add)
                else:
                    for kf in range(FC):
                        f1 = b2.tile([128, S], F32, tag="b2")
                        wmm(f1[:], l1, slice(kf * 128, (kf + 1) * 128), h2, False)
                        rt = rtp.tile([128, S], BF, tag="rt")
                        nc.scalar.activation(rt[:], f1[:], AF.Relu,
                                             bias=l1b[:, kf:kf + 1])
                        for m in range(KC):
                            nc.tensor.matmul(f2[m][:],
                                             l2[:, kf, m * 128:(m + 1) * 128],
                                             rt[:], start=(kf == 0),
                                             stop=(kf == FC - 1))
                    for m in range(KC):
                        nc.vector.scalar_tensor_tensor(
                            x[:, m, sl], f2[m][:], l2b[:, m:m + 1], x[:, m, sl],
                            ALU.add, ALU.add)
                rcA[s] = ln_stats(x, s)
            dump(f"dx{li + 1}", x[:])

        # ---------------- final encoder LN -> me (bf16) --------------------
        for s in range(BL):
            ln_apply(rcA[s], x, s, me[:, :, s * S:(s + 1) * S])
        dump("dme", me[:])
        dump("dp0", p[:])

        # ---------------- decoder layers ----------------
        for li in range(ld):
            dwq = w2pool.tile([128, KC, D], BF, tag="w2")
            nc.sync.dma_start(dwq[:], dram["dinqT"][li])
            dw = wpool.tile([128, KC, 2 * D], FP8, tag="bigw")
            nc.sync.dma_start(dw[:], dram["dinkvT"][li])
            dwb = bpool.tile([128, 12], F32, tag="w1b")
            nc.sync.dma_start(dwb[:], dram["dinb"][li])
            dvbrow = bpool.tile([1, D], F32, tag="vbrow")
            nc.sync.dma_start(dvbrow[:], dram["dvb"][li])
            do = owpool.tile([128, KC, D], BF, tag="ow")
            nc.sync.dma_start(do[:], dram["dowT"][li])
            dob = bpool.tile([128, 4], F32, tag="owb")
            nc.sync.dma_start(dob[:], dram["dowb"][li])
            m1 = wpool.tile([128, KC, M], BF, tag="bigw")
            nc.sync.dma_start(m1[:], dram["dm1T"][li])
            m1b = bpool.tile([128, 16], F32, tag="l1b")
            nc.sync.dma_start(m1b[:], dram["dm1b"][li])
            m2b = bpool.tile([128, 4], F32, tag="l2b")
            nc.sync.dma_start(m2b[:], dram["dm2b"][li])

            vb_bc = vbp.tile([128, D], F32, tag="vb_bc")
            nc.gpsimd.partition_broadcast(vb_bc[:], dvbrow[:])

            # LN(p) -> q_ln ; Q projection (all samples at once, N=BL)
            q_ln = smalls.tile([128, KC, BL], BF, tag="q_ln")
            ln(p, BL, q_ln)
            qps = b1.tile([128, KC, BL], F32, tag="b1")
            for m in range(KC):
                for k in range(KC):
                    nc.tensor.matmul(qps[:, m, :],
                                     dwq[:, k, m * 128:(m + 1) * 128],
                                     q_ln[:, k, :], start=(k == 0),
                                     stop=(k == KC - 1))
            q_sb = smalls.tile([128, KC, BL], BF, tag="q_sb")
            for m in range(KC):
                nc.vector.tensor_scalar(q_sb[:, m, :], qps[:, m, :],
                                        dwb[:, m:m + 1], None, ALU.add)
            o_d = smalls.tile([128, KC, BL], BF, tag="o_d")
            for s in range(BL):
                sl = slice(s * S, (s + 1) * S)
                # K (feature-major) and V' (token-major) over morph_enc
                k_sb = a4.tile([128, KC, S], BF, tag="a4")
                me_s = me[:, :, sl]
                for m in range(KC):
                    ps = b1.tile([128, S], F32, tag="b1")
                    wmm(ps[:], dw, slice(m * 128, (m + 1) * 128), me_s, FP8_DEC)
                    nc.vector.tensor_scalar(k_sb[:, m, :], ps[:], D_SC,
                                            dwb[:, 4 + m:5 + m], ALU.mult, ALU.add)
                vloc = vp.tile([128, KC, 8, 65], BF, tag="vloc")
                for t in range(KC):
                    nc.vector.tensor_copy(vloc[:, t, :, 64], ones8[:])
                for t in range(KC):
                    ps = b1.tile([128, S], F32, tag="b1")
                    tsl = slice(s * S + t * 128, s * S + (t + 1) * 128)
                    if FP8_DEC:
                        for kp in range(KC // 2):
                            nc.tensor.matmul(
                                ps[:], me[:, 2 * kp:2 * kp + 2, tsl],
                                dw[:, 2 * kp:2 * kp + 2, D:2 * D], perf_mode=DR,
                                start=(kp == 0), stop=(kp == KC // 2 - 1))
                    else:
                        for k in range(KC):
                            nc.tensor.matmul(
                                ps[:], me[:, k, tsl], dw[:, k, D:2 * D],
                                start=(k == 0), stop=(k == KC - 1))
                    nc.vector.scalar_tensor_tensor(
                        vloc[:, t, :, 0:64],
                        ps[:].rearrange("p (h d) -> p h d", h=H), D_SC,
                        vb_bc[:].rearrange("p (h d) -> p h d", h=H),
                        ALU.mult, ALU.add)
                scp = b1.tile([128, KC, H], F32, tag="b1")
                for hh in range(H):
                    rows = slice(64 * (hh % 2), 64 * (hh % 2) + 64)
                    for c in range(KC):
                        nc.tensor.matmul(
                            scp[:, c, hh:hh + 1],
                            k_sb[rows, hh // 2, c * 128:(c + 1) * 128],
                            q_sb[rows, hh // 2, s:s + 1],
                            start=True, stop=True)
                at = smalls.tile([128, KC, H], BF, tag="at_d")
                nc.scalar.activation(at[:], scp[:], AF.Exp,
                                     scale=float(1.0 / np.sqrt(DH)))
                ov = b2.tile([65, H], F32, tag="b2")
                for hh in range(H):
                    for c in range(KC):
                        nc.tensor.matmul(ov[:, hh:hh + 1], vloc[:, c, hh, :],
                                         at[:, c, hh:hh + 1],
                                         start=(c == 0), stop=(c == KC - 1))
                den_d = scr.tile([1, H], F32, tag="scr")
                nc.vector.tensor_copy(den_d[:], ov[64:65, :])
                rec = scr.tile([1, H], F32, tag="scr")
                nc.vector.reciprocal_approx_fast(rec[:], den_d[:])
                rb = scr.tile([64, H], F32, tag="scr_rb")
                nc.gpsimd.partition_broadcast(rb[:], rec[:])
                for hh in range(H):
                    rows = slice(64 * (hh % 2), 64 * (hh % 2) + 64)
                    nc.vector.tensor_tensor(o_d[rows, hh // 2, s:s + 1],
                                            ov[0:64, hh:hh + 1],
                                            rb[:, hh:hh + 1], ALU.mult)
            # out-proj + residual into p
            ops = b1.tile([128, KC, BL], F32, tag="b1")
            for m in range(KC):
                for k in range(KC):
                    nc.tensor.matmul(ops[:, m, :],
                                     do[:, k, m * 128:(m + 1) * 128],
                                     o_d[:, k, :], start=(k == 0),
                                     stop=(k == KC - 1))
            for m in range(KC):
                nc.vector.scalar_tensor_tensor(
                    p[:, m, :], ops[:, m, :], dob[:, m:m + 1], p[:, m, :],
                    ALU.add, ALU.add)
            # FFN on p (m2 streams in chunked once dw releases its slot)
            m2 = wpool.tile([128, MC, D], BF, tag="bigw")
            for kf in range(MC):
                nc.sync.dma_start(m2[:, kf, :], dram["dm2T"][li][:, kf, :])
            h2d = smalls.tile([128, KC, BL], BF, tag="q_ln")
            ln(p, BL, h2d)
            mh = smalls.tile([128, MC, BL], BF, tag="mh")
            for mm_ in range(MC):
                ps = b1.tile([128, BL], F32, tag="b1")
                for k in range(KC):
                    nc.tensor.matmul(ps[:], m1[:, k, mm_ * 128:(mm_ + 1) * 128],
                                     h2d[:, k, :], start=(k == 0),
                                     stop=(k == KC - 1))
                nc.scalar.activation(mh[:, mm_, :], ps[:], AF.Relu,
                                     bias=m1b[:, mm_:mm_ + 1])
            m2ps = b1.tile([128, KC, BL], F32, tag="b1")
            for m in range(KC):
                for kf in range(MC):
                    nc.tensor.matmul(m2ps[:, m, :],
                                     m2[:, kf, m * 128:(m + 1) * 128],
                                     mh[:, kf, :], start=(kf == 0),
                                     stop=(kf == MC - 1))
            for m in range(KC):
                nc.vector.scalar_tensor_tensor(
                    p[:, m, :], m2ps[:, m, :], m2b[:, m:m + 1], p[:, m, :],
                    ALU.add, ALU.add)
            dump(f"dp{li + 1}", p[:])

        # ---------------- head ----------------
        hw = smalls.tile([128, KC], BF, tag="hw")
        hb = smalls.tile([1, 1], F32, tag="hb")
        nc.sync.dma_start(hw[:], dram["hwT"][:])
        nc.sync.dma_start(hb[:], dram["hb"][:])
        hg = smalls.tile([128, KC, BL], BF, tag="q_ln")
        ln(p, BL, hg)
        hps = b2.tile([1, BL], F32, tag="b2")
        for k in range(KC):
            nc.tensor.matmul(hps[:], hw[:, k:k + 1], hg[:, k, :],
                             start=(k == 0), stop=(k == KC - 1))
        y_sb = smalls.tile([1, BL], F32, tag="y_sb")
        nc.scalar.activation(y_sb[:], hps[:], AF.Sigmoid, bias=hb[:])
        nc.sync.dma_start(y_dram[:], y_sb[:])


# ----------------------------------------------------------------------------
# entry point
# ----------------------------------------------------------------------------

_NC_CACHE = {}


def kernel(**inputs):
    return _run(inputs, LE, LD)


def _run(inputs, le, ld, trace=False):
    w = prep_weights(inputs, le, ld)
    morph = np.asarray(inputs["morph"], np.float32)
    pose = np.asarray(inputs["pose"], np.float32)
    in_maps = []
    for c in range(NCORES):
        im = dict(w)
        mo = morph[c * BL:(c + 1) * BL]                 # [BL, S, 3]
        im["morphT"] = np.ascontiguousarray(
            mo.transpose(2, 0, 1).reshape(3, T))
        im["poseT"] = np.ascontiguousarray(pose[c * BL:(c + 1) * BL].T)
        in_maps.append(im)

    if ("nc", le, ld) not in _NC_CACHE:
        _NC_CACHE[("nc", le, ld)] = build(le, ld)
    nc = _NC_CACHE[("nc", le, ld)]
    res = run_bass_kernel_spmd(nc, in_maps, core_ids=list(range(NCORES)),
                               trace=trace)
    out = np.zeros((B, 1), np.float32)
    for c in range(NCORES):
        out[c * BL:(c + 1) * BL, 0] = res.results[c]["y"][0]
    if trace:
        return out, res
    return out
